# revision 4
# baseline (speedup 1.0000x reference)
"""Batched per-item k-means (Lloyd) kernel for Trainium2, 8-core data parallel.

Problem: features [32, 4096, 256] f32 -> L2-normalize rows, then per batch item
run K=24-means for 25 iterations (deterministic linspace init), return
(centers [32,24,256], slot_masks [32,24,4096]).

Sharding: batch dim across 8 cores (4 items/core), no collectives.

Per-core per-item pipeline (all on-chip after one x load):
  - x stored twice: x_aug [n_part, tile, d(+ones)] and x_DN [d_part, half, n]
    (built by PE transposes).
  - distance phase: centers_DK as stationary, x_DN moving ->
    dot [24, n] in PSUM; ScalarE fuses d' = -2*dot + c_sq while copying to
    SBUF; PE transposes to [n_part, k]; VectorE argmin via
    reduce_min / eq / mul-rev-iota / reduce_max / eq chain -> exact
    first-min one-hot.
  - scatter phase: onehot tiles stationary, x_aug (with ones column) moving ->
    sums[24, 256] and counts in one PSUM accumulation.
  - update: centers = sums * recip(max(counts,1)) + (counts==0)*old,
    c_sq via ScalarE Square+accum, PE transposes back to DK layout.
Matmuls are fp32 (argmin flips appear below ~1e-6 relative matmul error;
bf16/tf32-class precision is fatal for matching the fp32 reference).
"""

import numpy as np
from contextlib import ExitStack

B_FULL = 32
N_CORES = 8
B_LOC = B_FULL // N_CORES
N_FULL = 4096
D = 256
K = 24
N_ITERS = 25
NORM_EPS = 1e-12

_CACHE = {}


def _build_nc(b_loc=B_LOC, n=N_FULL, n_iters=N_ITERS, mm_dtype_name="float32"):
    import concourse.bass as bass
    import concourse.tile as tile
    from concourse import bacc, mybir

    f32 = mybir.dt.float32
    i32 = mybir.dt.int32
    mm_dt = getattr(mybir.dt, mm_dtype_name)
    Alu = mybir.AluOpType
    Act = mybir.ActivationFunctionType
    X = mybir.AxisListType.X

    NT = n // 128           # n-tiles of 128 points
    CH = n // 512           # 512-wide distance chunks
    XA_W = 264              # x_aug row stride (256 d + 1 ones + pad)
    init_idx = np.linspace(0, n - 1, K, dtype=np.float32).astype(np.int32)

    nc = bacc.Bacc()
    feat = nc.declare_dram_parameter("features", [b_loc, n, D], f32, isOutput=False)
    out_c = nc.declare_dram_parameter("centers", [b_loc, K, D], f32, isOutput=True)
    out_m = nc.declare_dram_parameter("slot_masks", [b_loc, K, n], f32, isOutput=True)

    def mmv(ap):
        # view an fp32 AP as the matmul dtype (no-op for float32)
        return ap if mm_dtype_name == "float32" else ap.bitcast(mm_dt)

    with tile.TileContext(nc) as tc, ExitStack() as ctx:
        P = lambda name, bufs, **kw: ctx.enter_context(
            tc.tile_pool(name=name, bufs=bufs, **kw)
        )
        const_p = P("const", 1)
        xaug_p = P("xaug", 2)
        xdn_p = P("xdn", 2)
        ckd_p = P("ckd", 3)
        cdk_p = P("cdk", 2)
        csq_p = P("csq", 2)
        small_p = P("small", 2)
        dsb_p = P("dsb", 1)
        dnk_p = P("dnk", 2)
        amin_p = P("amin", 2)
        oh_p = P("oh", 2)
        scr_p = P("scr", 1)
        msk_p = P("msk", 1)
        # PSUM pools (each tile rounds up to a 2KB bank; 8 banks total)
        dist_ps = P("dist_ps", 2, space="PSUM")
        dT_ps = P("dT_ps", 2, space="PSUM")
        sums_ps = P("sums_ps", 1, space="PSUM")
        xT_ps = P("xT_ps", 2, space="PSUM")

        # constants
        ident = const_p.tile([128, 128], f32)
        nc.gpsimd.memset(ident[:, :], 1.0)
        nc.gpsimd.affine_select(
            ident[:, :], ident[:, :], pattern=[[-1, 128]],
            compare_op=Alu.is_equal, fill=0.0, base=0, channel_multiplier=1,
        )
        rev_i = const_p.tile([128, K], i32)
        nc.gpsimd.iota(rev_i[:, :], pattern=[[-1, K]], base=K, channel_multiplier=0)
        rev_f = const_p.tile([128, K], f32)
        nc.vector.tensor_copy(rev_f[:, :], rev_i[:, :])

        for b in range(b_loc):
            # ---------------- per-item load & setup ----------------
            x_aug = xaug_p.tile([128, NT * XA_W], f32)
            xa3 = x_aug[:, :].rearrange("p (t w) -> p t w", w=XA_W)
            src = feat[b].rearrange("(t p) d -> p t d", p=128)
            for g in range(4):
                nc.sync.dma_start(
                    out=xa3[:, g * (NT // 4):(g + 1) * (NT // 4), 0:D],
                    in_=src[:, g * (NT // 4):(g + 1) * (NT // 4), :],
                )
            nc.gpsimd.memset(xa3[:, :, D:D + 1], 1.0)

            ckd = ckd_p.tile([K, D], f32, tag="ckd")
            for k in range(K):
                nc.sync.dma_start(out=ckd[k:k + 1, :], in_=feat[b, int(init_idx[k]), :][None, :])

            # x_DN: [d_part, half, n] via PE transposes
            x_dn = xdn_p.tile([128, 2 * n], f32)
            xdn3 = x_dn[:, :].rearrange("p (h m) -> p h m", h=2)
            for t in range(NT):
                for h in range(2):
                    xT = xT_ps.tile([128, 128], f32, tag="xT")
                    nc.tensor.transpose(xT[:, :], xa3[:, t, h * 128:(h + 1) * 128], ident[:, :])
                    eng = nc.vector if (t + h) % 2 == 0 else nc.scalar
                    if eng is nc.vector:
                        nc.vector.tensor_copy(xdn3[:, h, t * 128:(t + 1) * 128], xT[:, :])
                    else:
                        nc.scalar.copy(xdn3[:, h, t * 128:(t + 1) * 128], xT[:, :])

            # centers_DK + c_sq for iteration 0
            cdk = cdk_p.tile([128, 2 * K], f32, tag="cdk")
            cdk3 = cdk[:, :].rearrange("p (h k) -> p h k", h=2)
            cT = xT_ps.tile([128, 2 * K], f32, tag="xT")
            for h in range(2):
                nc.tensor.transpose(cT[:, h * K:(h + 1) * K], ckd[:, h * 128:(h + 1) * 128], ident[0:K, 0:K])
            nc.vector.tensor_copy(cdk[:, :], cT[:, :])
            csq = csq_p.tile([K, 1], f32, tag="csq")
            sq_scr = scr_p.tile([K, D], f32)
            nc.scalar.activation(sq_scr[:, :], ckd[:, :], Act.Square, accum_out=csq[:, :])

            # ---------------- iterations ----------------
            for it in range(n_iters + 1):
                last = it == n_iters
                # Phase D+C: dot -> d' = -2*dot + c_sq in dist_sb
                dist_sb = dsb_p.tile([K, n], f32)
                for c in range(CH):
                    dps = dist_ps.tile([K, 512], f32, tag="dist")
                    for h in range(2):
                        nc.tensor.matmul(
                            dps[:, :], mmv(cdk3[:, h, :]),
                            mmv(xdn3[:, h, c * 512:(c + 1) * 512]),
                            start=(h == 0), stop=(h == 1),
                        )
                    nc.scalar.activation(
                        dist_sb[:, c * 512:(c + 1) * 512], dps[:, :],
                        Act.Identity, bias=csq[:, :], scale=-2.0,
                    )
                # transposes to [n_part, k]
                dist_nk = dnk_p.tile([128, NT * K], f32)
                for g in range(NT // 4):
                    dT = dT_ps.tile([128, 4 * K], f32, tag="dT")
                    for j in range(4):
                        t = g * 4 + j
                        nc.tensor.transpose(
                            dT[:, j * K:(j + 1) * K],
                            dist_sb[:, t * 128:(t + 1) * 128], ident[0:K, 0:K],
                        )
                    nc.vector.tensor_copy(dist_nk[:, g * 4 * K:(g + 1) * 4 * K], dT[:, :])
                # Phase A: exact first-min one-hot
                d3 = dist_nk[:, :].rearrange("p (t k) -> p t k", k=K)
                min_d = small_p.tile([128, NT], f32, tag="min_d")
                nc.vector.tensor_reduce(min_d[:, :].unsqueeze(2), d3, axis=X, op=Alu.min)
                is_min = amin_p.tile([128, NT * K], f32, tag="is_min")
                im3 = is_min[:, :].rearrange("p (t k) -> p t k", k=K)
                nc.vector.tensor_tensor(
                    im3, d3, min_d[:, :].unsqueeze(2).broadcast_to((128, NT, K)),
                    op=Alu.is_equal,
                )
                enc = amin_p.tile([128, NT * K], f32, tag="enc")
                e3 = enc[:, :].rearrange("p (t k) -> p t k", k=K)
                nc.vector.tensor_tensor(
                    e3, im3, rev_f[:, :].unsqueeze(1).broadcast_to((128, NT, K)),
                    op=Alu.mult,
                )
                emax = small_p.tile([128, NT], f32, tag="emax")
                nc.vector.tensor_reduce(emax[:, :].unsqueeze(2), e3, axis=X, op=Alu.max)
                onehot = oh_p.tile([128, NT * K], f32, tag="onehot")
                oh3 = onehot[:, :].rearrange("p (t k) -> p t k", k=K)
                nc.vector.tensor_tensor(
                    oh3, e3, emax[:, :].unsqueeze(2).broadcast_to((128, NT, K)),
                    op=Alu.is_equal,
                )

                if last:
                    # slot_masks = onehot^T via PE transposes, then out
                    masks = msk_p.tile([K, n], f32)
                    for g in range(NT // 4):
                        mT = dist_ps.tile([K, 512], f32, tag="dist")
                        for j in range(4):
                            t = g * 4 + j
                            nc.tensor.transpose(
                                mT[:, j * 128:(j + 1) * 128], oh3[:, t, :], ident[:, :],
                            )
                        nc.scalar.copy(masks[:, g * 512:(g + 1) * 512], mT[:, :])
                    nc.sync.dma_start(out=out_m[b], in_=masks[:, :])
                    nc.sync.dma_start(out=out_c[b], in_=ckd[:, :])
                    break

                # Phase S: scatter-sum + counts in one PSUM accumulation
                sps = sums_ps.tile([K, D + 1], f32, tag="sums")
                for t in range(NT):
                    nc.tensor.matmul(
                        sps[:, :], mmv(oh3[:, t, :]), mmv(xa3[:, t, 0:D + 1]),
                        start=(t == 0), stop=(t == NT - 1),
                    )
                # Phase U: centers update
                counts = sps[:, D:D + 1]
                cc = small_p.tile([K, 1], f32, tag="cc")
                nc.vector.tensor_single_scalar(cc[:, :], counts, 1.0, op=Alu.max)
                recip = small_p.tile([K, 1], f32, tag="recip")
                nc.vector.reciprocal(recip[:, :], cc[:, :])
                is_emp = small_p.tile([K, 1], f32, tag="is_emp")
                nc.vector.tensor_single_scalar(is_emp[:, :], counts, 0.0, op=Alu.is_equal)
                newkd = scr_p.tile([K, D], f32, tag="newkd")
                nc.vector.tensor_scalar_mul(newkd[:, :], sps[:, 0:D], recip[:, :])
                ckd_new = ckd_p.tile([K, D], f32, tag="ckd")
                nc.vector.scalar_tensor_tensor(
                    ckd_new[:, :], ckd[:, :], is_emp[:, :], newkd[:, :],
                    op0=Alu.mult, op1=Alu.add,
                )
                ckd = ckd_new
                cdk = cdk_p.tile([128, 2 * K], f32, tag="cdk")
                cdk3 = cdk[:, :].rearrange("p (h k) -> p h k", h=2)
                cT = xT_ps.tile([128, 2 * K], f32, tag="xT")
                for h in range(2):
                    nc.tensor.transpose(cT[:, h * K:(h + 1) * K], ckd[:, h * 128:(h + 1) * 128], ident[0:K, 0:K])
                nc.vector.tensor_copy(cdk[:, :], cT[:, :])
                csq = csq_p.tile([K, 1], f32, tag="csq")
                sq_scr2 = scr_p.tile([K, D], f32)
                nc.scalar.activation(sq_scr2[:, :], ckd[:, :], Act.Square, accum_out=csq[:, :])
    nc.compile()
    return nc


def _get_nc(key=("full", "float32")):
    if key not in _CACHE:
        kind, mmdt = key
        if kind == "full":
            _CACHE[key] = _build_nc(B_LOC, N_FULL, N_ITERS, mmdt)
        else:
            _CACHE[key] = _build_nc(1, 512, 2, mmdt)
    return _CACHE[key]


def _normalize(features):
    x = np.ascontiguousarray(features, dtype=np.float32)
    sq = np.einsum("bnd,bnd->bn", x, x, dtype=np.float32)
    norm = np.sqrt(sq, dtype=np.float32)[..., None]
    return (x / np.maximum(norm, np.float32(NORM_EPS))).astype(np.float32)


def run(features, trace=False, mm_dtype="float32"):
    from concourse.bass_utils import run_bass_kernel_spmd

    f = _normalize(features)
    nc = _get_nc(("full", mm_dtype))
    in_maps = [
        {"features": f[m * B_LOC:(m + 1) * B_LOC]} for m in range(N_CORES)
    ]
    res = run_bass_kernel_spmd(nc, in_maps, list(range(N_CORES)), trace=trace)
    centers = np.concatenate([np.asarray(r["centers"]) for r in res.results], axis=0)
    masks = np.concatenate([np.asarray(r["slot_masks"]) for r in res.results], axis=0)
    return (centers, masks), res


def kernel(features):
    (centers, masks), _ = run(features, trace=False)
    return centers, masks
